# revision 11
# baseline (speedup 1.0000x reference)
"""Optimized Trainium2 Bass kernel for nn_KernelNet2d (dense_mlp, memory regime).

Network: pos(1,2,64,64) -> 1x1convs 2->16->16->4->16384, leaky_relu(0.1)
between layers, output reshaped to (128,128,64,64).

Sharding: conv4's 16384 output channels split across 8 cores (2048 each);
early layers replicated.

Key optimizations over the fp32 baseline (~188us -> ~65us simulated):
  - Output stored as fp16 (rel err ~5e-4 vs fp32 reference): halves HBM
    write traffic 32MB -> 16MB per core, the dominant cost.
  - ALL matmuls in fp32r (TF32-style PE mode): 1 cycle/row at N>=256 vs
    4 cycles/row for plain fp32; layer 4 is a single K=5 matmul
    (4 weights + folded bias) per (128ch x 512px) tile, ~2e-4 accurate.
    fp32r output must start at partition 0, so early-layer pixel chunks
    sit in COLUMNS of multi-bank psum tiles.
  - Leaky-relu as ONE ACT op: activation(Prelu, alpha=0.1) reading PSUM
    and writing f32r directly (the Lrelu LUT hardcodes slope 0.01, and
    scalar_tensor_tensor fails BIR verification in this walrus build).
  - No memsets: biases fold in as weight rows against a ones row, and
    each layer re-emits the next ones row via an extra weight column
    (ones survive lrelu); inputs packed into 3 DRAM tensors to cut
    serialized DMA-issue latency at the start.
  - Dummy matmuls during the input loads hold the PE p-state ramp so the
    first real matmul runs at full clock.
  - Early layers software-pipelined (skew-1 wavefront over progressive
    512/1024-px groups) with tile-0's layer-4 interleaved, so the first
    HBM store issues at ~12us; the wavefront tail threads between the
    remaining tiles so the in-order PE never blocks.
  - PSUM evacuation [128,1024] copies alternate DVE/ACT (gpsimd cannot
    read PSUM on TRN2); quarter-tile stores from the idle SP queue keep
    the store stream dense (HWDGE generation ~625ns/DMA caps the count).
"""

import numpy as np

X_DIM, Y_DIM = 64, 64
NPIX = X_DIM * Y_DIM          # 4096
CH_OUT, CH_IN = 128, 128
NUM_C = CH_OUT * CH_IN        # 16384
N_CORES = 8
CPC = NUM_C // N_CORES        # 2048 channels per core
NEG_SLOPE = 0.1

PX_CHUNK = 512
N_CHUNK = NPIX // PX_CHUNK    # 8
N_GRP = 2                     # chunk groups of 4 (one per PE quadrant)
CH_TILE = 128
N_TILE = CPC // CH_TILE       # 16
STORE_SPLIT = 2               # stores per output tile

_COMPILED = {}


def _build_nc(reps=1):
    from concourse import bacc, mybir, tile

    f32 = mybir.dt.float32
    f32r = mybir.dt.float32r
    f16 = mybir.dt.float16
    AF = mybir.ActivationFunctionType

    nc = bacc.Bacc(trn_type="TRN2", target_bir_lowering=False)

    # pos + ones row + layer-1 weights packed in one tensor; layer-2/3
    # weights in another: fewer serialized input DMAs at start.
    in1_d = nc.dram_tensor("in1", [3, NPIX + 17], f32r, kind="ExternalInput")
    w23_d = nc.dram_tensor("w23", [17, 22], f32r, kind="ExternalInput")
    w4q_d = nc.dram_tensor("w4q", [5, CPC], f32r, kind="ExternalInput")
    out_d = nc.dram_tensor("out", [CPC, NPIX], f16, kind="ExternalOutput")

    EC = 2 * PX_CHUNK            # early-layer psum tile width (2 banks)
    N_EG = NPIX // EC            # 4 psum groups per early layer

    with tile.TileContext(nc) as tc:
        with (
            tc.tile_pool(name="persist", bufs=1) as per,
            tc.tile_pool(name="ps4", bufs=4, space="PSUM") as ps4p,
            tc.tile_pool(name="stage", bufs=3) as stg,
        ):
            in1_t = per.tile([3, NPIX + 17], f32r)
            w23_t = per.tile([17, 22], f32r)
            w4_t = per.tile([5, CPC], f32r)
            dmw = per.tile([1, 128], f32r)
            dmx = per.tile([1, PX_CHUNK], f32r)
            dm32 = per.tile([1, PX_CHUNK], f32)
            x1 = per.tile([17, NPIX], f32r)   # rows 0-15 acts, row 16 ones
            x2 = per.tile([17, NPIX], f32r)
            x3 = per.tile([5, NPIX], f32r)    # rows 0-3 acts, row 4 ones

            nc.sync.dma_start(in1_t[:], in1_d[:])
            nc.sync.dma_start(w23_t[:], w23_d[:])
            nc.sync.dma_start(w4_t[:], w4q_d[:])

            # warm up the PE during the input loads: garbage matmuls hold
            # the p-state ramp so the first real matmul runs at full clock
            nc.vector.memset(dm32[:], 0.0)
            nc.vector.tensor_copy(dmw[:], dm32[:, 0:128])
            nc.vector.tensor_copy(dmx[:], dm32[:])
            for i in range(4):
                psd = ps4p.tile([CH_TILE, 2 * PX_CHUNK], f32, tag="ps4",
                                name=f"psd{i}")
                nc.tensor.matmul(psd[:, 0:PX_CHUNK], dmw[:], dmx[:],
                                 skip_group_check=True)

            for _ in range(reps):
                # ---- early layers 2->16->16->4 (lrelu via ACT Prelu with
                # alpha=0.1) + tile-0 layer 4, software-pipelined with
                # skew 1 over 1024-px groups: PE batches each stage while
                # ACT runs one stage behind, and tile-0 stores issue while
                # later groups are still in flight. fp32r matmul output
                # must start at partition 0, so chunks sit in COLUMNS of
                # 2-bank psum tiles.
                lay = (
                    (x1, None, in1_t[:, NPIX : NPIX + 17], 17),
                    (x2, x1, w23_t[:, 0:17], 17),
                    (x3, x2, w23_t[:, 17:22], 5),
                )
                seg = N_CHUNK // STORE_SPLIT
                st0 = stg.tile([CH_TILE, NPIX], f16, tag="st", name="st0")
                # progressive pixel groups: 4x512 first so tile-0's first
                # store only waits on small Prelus, then 2x1024
                GRP = [(0, 512), (512, 512), (1024, 512), (1536, 512),
                       (2048, 1024), (3072, 1024)]
                ST0 = {1: (0, 1024), 3: (1024, 1024), 5: (2048, 2048)}
                NG = len(GRP)

                def early(snum, g):
                    xo, xi, wap, rows = lay[snum]
                    g0, gw = GRP[g]
                    pse = ps4p.tile([CH_TILE, EC], f32, tag="ps4",
                                    name=f"pse{snum}_{g}")
                    for h in range(gw // PX_CHUNK):
                        cs = slice(g0 + h * PX_CHUNK,
                                   g0 + (h + 1) * PX_CHUNK)
                        rhs = in1_t[:, cs] if xi is None else xi[:, cs]
                        nc.tensor.matmul(
                            pse[0:rows, h * PX_CHUNK : (h + 1) * PX_CHUNK],
                            wap, rhs, skip_group_check=True)
                    nc.scalar.activation(xo[:, g0 : g0 + gw],
                                         pse[0:rows, 0:gw],
                                         AF.Prelu, alpha=NEG_SLOPE)

                def l4tile0(g):
                    g0, gw = GRP[g]
                    ps = ps4p.tile([CH_TILE, EC], f32, tag="ps4",
                                   name=f"ps40_{g}")
                    for h in range(gw // PX_CHUNK):
                        cs = slice(g0 + h * PX_CHUNK,
                                   g0 + (h + 1) * PX_CHUNK)
                        nc.tensor.matmul(
                            ps[:, h * PX_CHUNK : (h + 1) * PX_CHUNK],
                            w4_t[:, 0:CH_TILE], x3[:, cs],
                            skip_group_check=True)
                    nc.vector.tensor_copy(st0[:, g0 : g0 + gw],
                                          ps[:, 0:gw])
                    if g in ST0:
                        s0, sw = ST0[g]
                        nc.sync.dma_start(out_d[0:CH_TILE, s0 : s0 + sw],
                                          st0[:, s0 : s0 + sw])

                defer = []
                for step in range(4 + NG - 1):
                    for snum in range(4):
                        g = step - snum
                        if 0 <= g < NG:
                            if g >= 4:
                                defer.append((snum, g))
                            elif snum < 3:
                                early(snum, g)
                            else:
                                l4tile0(g)

                # ---- layer 4, tiles 1-15 (tail of the prologue wavefront
                # threaded between the first tiles so the PE never blocks)
                evac = 0
                for t in range(1, N_TILE):
                    if defer:
                        snum, e = defer.pop(0)
                        if snum < 3:
                            early(snum, e)
                        else:
                            l4tile0(e)
                    ts = slice(t * CH_TILE, (t + 1) * CH_TILE)
                    st = stg.tile([CH_TILE, NPIX], f16, tag="st")
                    for cc in range(N_CHUNK // 2):
                        ps = ps4p.tile([CH_TILE, 2 * PX_CHUNK], f32,
                                       tag="ps4")
                        for h in range(2):
                            c = 2 * cc + h
                            cs = slice(c * PX_CHUNK, (c + 1) * PX_CHUNK)
                            nc.tensor.matmul(
                                ps[:, h * PX_CHUNK : (h + 1) * PX_CHUNK],
                                w4_t[:, ts], x3[:, cs],
                                skip_group_check=True)
                        cs2 = slice(2 * cc * PX_CHUNK,
                                    (2 * cc + 2) * PX_CHUNK)
                        eng = nc.scalar if (evac * 28) // 60 != \
                            ((evac - 1) * 28) // 60 else nc.vector
                        evac += 1
                        if eng is nc.scalar:
                            eng.copy(st[:, cs2], ps[:])
                        else:
                            eng.tensor_copy(st[:, cs2], ps[:])
                        if (2 * cc + 2) % seg == 0:
                            ss = slice((2 * cc + 2 - seg) * PX_CHUNK,
                                       (2 * cc + 2) * PX_CHUNK)
                            nc.sync.dma_start(out_d[ts, ss], st[:, ss])

    nc.compile()
    return nc


def _get_nc():
    if "nc" not in _COMPILED:
        _COMPILED["nc"] = _build_nc()
    return _COMPILED["nc"]


def _make_in_maps(pos, w1, b1, w2, b2, w3, b3, w4, b4):
    f32 = np.float32

    # in1: pos rows 0-1, ones row 2; cols 4096.. hold w1q [3, 17]
    # (w1.T + bias row, col 16 emits the ones row for the next layer)
    in1 = np.ones((3, NPIX + 17), f32)
    in1[0:2, 0:NPIX] = np.asarray(pos, f32).reshape(2, NPIX)
    w1q = np.zeros((3, 17), f32)
    w1q[0:2, 0:16] = np.asarray(w1, f32).T
    w1q[2, 0:16] = np.asarray(b1, f32)
    w1q[2, 16] = 1.0
    in1[:, NPIX:] = w1q

    # w23: cols 0-16 layer-2 lhsT, cols 17-21 layer-3 lhsT; bias row 16
    # pairs the ones row, last col emits the next ones row
    w23 = np.zeros((17, 22), f32)
    w23[0:16, 0:16] = np.asarray(w2, f32).T
    w23[16, 0:16] = np.asarray(b2, f32)
    w23[16, 16] = 1.0
    w23[0:16, 17:21] = np.asarray(w3, f32).T
    w23[16, 17:21] = np.asarray(b3, f32)
    w23[16, 21] = 1.0

    w4 = np.asarray(w4, f32)
    b4 = np.asarray(b4, f32)

    in_maps = []
    for i in range(N_CORES):
        w4q = np.zeros((5, CPC), f32)
        w4q[0:4] = w4[i * CPC : (i + 1) * CPC, :].T
        w4q[4] = b4[i * CPC : (i + 1) * CPC]
        in_maps.append({"in1": in1, "w23": w23, "w4q": w4q})
    return in_maps


def _run_pjrt_nodonate(nc, in_maps):
    """Execute via PJRT WITHOUT output-buffer donation.

    The stock run_bass_via_pjrt donates pre-zeroed output buffers; on this
    axon terminal the donated path intermittently returns corrupted
    outputs, while the identical executable without donation is reliable.
    The kernel writes every output element, so donation is unnecessary.
    """
    import jax
    from jax.sharding import Mesh, PartitionSpec, NamedSharding
    from jax.experimental.shard_map import shard_map
    from concourse import bass2jax, mybir
    from concourse.bass2jax import _bass_exec_p, partition_id_tensor

    bass2jax.install_neuronx_cc_hook()
    partition_name = (nc.partition_id_tensor.name
                      if nc.partition_id_tensor else None)
    in_names, out_names, out_avals = [], [], []
    for alloc in nc.m.functions[0].allocations:
        if not isinstance(alloc, mybir.MemoryLocationSet):
            continue
        name = alloc.memorylocations[0].name
        if alloc.kind == "ExternalInput":
            if name != partition_name:
                in_names.append(name)
        elif alloc.kind == "ExternalOutput":
            out_names.append(name)
            out_avals.append(jax.core.ShapedArray(
                tuple(alloc.tensor_shape), mybir.dt.np(alloc.dtype)))
    n_params = len(in_names)
    all_in = list(in_names) + list(out_names)
    if partition_name is not None:
        all_in.append(partition_name)

    def _body(*args):
        operands = list(args)
        if partition_name is not None:
            operands.append(partition_id_tensor())
        return tuple(_bass_exec_p.bind(
            *operands, out_avals=tuple(out_avals), in_names=tuple(all_in),
            out_names=tuple(out_names), lowering_input_output_aliases=(),
            sim_require_finite=True, sim_require_nnan=True, nc=nc))

    n = len(in_maps)
    mesh = Mesh(np.asarray(jax.devices()[:n]), ("core",))
    spec = (PartitionSpec("core"),)
    fn = jax.jit(shard_map(_body, mesh=mesh,
                           in_specs=spec * (n_params + len(out_names)),
                           out_specs=spec * len(out_names), check_rep=False),
                 keep_unused=True)
    args = [np.concatenate([np.asarray(m[nm]) for m in in_maps], axis=0)
            for nm in in_names]
    args += [np.zeros((n * av.shape[0], *av.shape[1:]), av.dtype)
             for av in out_avals]
    outs = fn(*args)
    return [
        {nm: np.asarray(outs[i]).reshape(n, *out_avals[i].shape)[c]
         for i, nm in enumerate(out_names)}
        for c in range(n)
    ]


def run(trace=False, tmpdir=None, **inputs):
    nc = _get_nc()
    in_maps = _make_in_maps(**inputs)
    full = results = None
    for _ in range(4):
        results = _run_pjrt_nodonate(nc, in_maps)
        parts = [np.asarray(results[i]["out"]) for i in range(N_CORES)]
        full = np.concatenate(parts, axis=0).reshape(
            CH_OUT, CH_IN, X_DIM, Y_DIM).astype(np.float32)
        # guard against the flaky first NEFF execution on a freshly
        # attached terminal (first call occasionally returns garbage on
        # some cores; subsequent executions are correct)
        if _check_row(inputs, full) < 5e-3:
            break

    class _Res:
        exec_time_ns = None
        results = None
    r = _Res()
    r.results = results
    return full, r


def _check_row(inputs, full):
    # cheap host recomputation of output channel (0,0): guards against a
    # flaky first NEFF execution on a freshly-attached terminal (observed:
    # first call occasionally returns garbage; later calls are correct)
    def lrelu(v):
        return np.where(v >= 0, v, NEG_SLOPE * v)

    pos = np.asarray(inputs["pos"], np.float32).reshape(2, NPIX)
    x = lrelu(inputs["w1"] @ pos + np.asarray(inputs["b1"])[:, None])
    x = lrelu(inputs["w2"] @ x + np.asarray(inputs["b2"])[:, None])
    x = lrelu(inputs["w3"] @ x + np.asarray(inputs["b3"])[:, None])
    w4 = np.asarray(inputs["w4"])
    b4 = np.asarray(inputs["b4"])
    outf = full.reshape(NUM_C, NPIX)
    # one sampled row per 128-channel tile on every core (vectorized)
    rows = w4[::CH_TILE] @ x + b4[::CH_TILE][:, None]
    denom = np.maximum(np.max(np.abs(rows), axis=1, keepdims=True), 1e-6)
    return float(np.max(np.abs(outf[::CH_TILE] - rows) / denom))


def kernel(**inputs: np.ndarray) -> np.ndarray:
    out, _ = run(trace=False, **inputs)
    return out


# revision 12
# speedup vs baseline: 1.0147x; 1.0147x over previous
"""Optimized Trainium2 Bass kernel for nn_KernelNet2d (dense_mlp, memory regime).

Network: pos(1,2,64,64) -> 1x1convs 2->16->16->4->16384, leaky_relu(0.1)
between layers, output reshaped to (128,128,64,64).

Sharding: conv4's 16384 output channels split across 8 cores (2048 each);
early layers replicated.

Key optimizations over the fp32 baseline (~188us -> ~65us simulated):
  - Output stored as fp16 (rel err ~5e-4 vs fp32 reference): halves HBM
    write traffic 32MB -> 16MB per core, the dominant cost.
  - ALL matmuls in fp32r (TF32-style PE mode): 1 cycle/row at N>=256 vs
    4 cycles/row for plain fp32; layer 4 is a single K=5 matmul
    (4 weights + folded bias) per (128ch x 512px) tile, ~2e-4 accurate.
    fp32r output must start at partition 0, so early-layer pixel chunks
    sit in COLUMNS of multi-bank psum tiles.
  - Leaky-relu as ONE ACT op: activation(Prelu, alpha=0.1) reading PSUM
    and writing f32r directly (the Lrelu LUT hardcodes slope 0.01, and
    scalar_tensor_tensor fails BIR verification in this walrus build).
  - No memsets: biases fold in as weight rows against a ones row, and
    each layer re-emits the next ones row via an extra weight column
    (ones survive lrelu); inputs packed into 3 DRAM tensors to cut
    serialized DMA-issue latency at the start.
  - Dummy matmuls during the input loads hold the PE p-state ramp so the
    first real matmul runs at full clock.
  - Early layers software-pipelined (skew-1 wavefront over progressive
    512/1024-px groups) with tile-0's layer-4 interleaved, so the first
    HBM store issues at ~12us; the wavefront tail threads between the
    remaining tiles so the in-order PE never blocks.
  - PSUM evacuation [128,1024] copies alternate DVE/ACT (gpsimd cannot
    read PSUM on TRN2); quarter-tile stores from the idle SP queue keep
    the store stream dense (HWDGE generation ~625ns/DMA caps the count).
"""

import numpy as np

X_DIM, Y_DIM = 64, 64
NPIX = X_DIM * Y_DIM          # 4096
CH_OUT, CH_IN = 128, 128
NUM_C = CH_OUT * CH_IN        # 16384
N_CORES = 8
CPC = NUM_C // N_CORES        # 2048 channels per core
NEG_SLOPE = 0.1

PX_CHUNK = 512
N_CHUNK = NPIX // PX_CHUNK    # 8
N_GRP = 2                     # chunk groups of 4 (one per PE quadrant)
CH_TILE = 128
N_TILE = CPC // CH_TILE       # 16
STORE_SPLIT = 4               # stores per output tile

_COMPILED = {}


def _build_nc(reps=1):
    from concourse import bacc, mybir, tile

    f32 = mybir.dt.float32
    f32r = mybir.dt.float32r
    f16 = mybir.dt.float16
    AF = mybir.ActivationFunctionType

    nc = bacc.Bacc(trn_type="TRN2", target_bir_lowering=False)

    # pos + ones row + layer-1 weights packed in one tensor; layer-2/3
    # weights in another: fewer serialized input DMAs at start.
    in1_d = nc.dram_tensor("in1", [3, NPIX + 17], f32r, kind="ExternalInput")
    w23_d = nc.dram_tensor("w23", [17, 22], f32r, kind="ExternalInput")
    w4q_d = nc.dram_tensor("w4q", [5, CPC], f32r, kind="ExternalInput")
    out_d = nc.dram_tensor("out", [CPC, NPIX], f16, kind="ExternalOutput")

    EC = 2 * PX_CHUNK            # early-layer psum tile width (2 banks)
    N_EG = NPIX // EC            # 4 psum groups per early layer

    with tile.TileContext(nc) as tc:
        with (
            tc.tile_pool(name="persist", bufs=1) as per,
            tc.tile_pool(name="ps4", bufs=4, space="PSUM") as ps4p,
            tc.tile_pool(name="stage", bufs=3) as stg,
        ):
            in1_t = per.tile([3, NPIX + 17], f32r)
            w23_t = per.tile([17, 22], f32r)
            w4_t = per.tile([5, CPC], f32r)
            dmw = per.tile([1, 128], f32r)
            dmx = per.tile([1, PX_CHUNK], f32r)
            dm32 = per.tile([1, PX_CHUNK], f32)
            x1 = per.tile([17, NPIX], f32r)   # rows 0-15 acts, row 16 ones
            x2 = per.tile([17, NPIX], f32r)
            x3 = per.tile([5, NPIX], f32r)    # rows 0-3 acts, row 4 ones

            nc.sync.dma_start(in1_t[:], in1_d[:])
            nc.sync.dma_start(w23_t[:], w23_d[:])
            nc.sync.dma_start(w4_t[:], w4q_d[:])

            # warm up the PE during the input loads: garbage matmuls hold
            # the p-state ramp so the first real matmul runs at full clock
            nc.vector.memset(dm32[:], 0.0)
            nc.vector.tensor_copy(dmw[:], dm32[:, 0:128])
            nc.vector.tensor_copy(dmx[:], dm32[:])
            for i in range(4):
                psd = ps4p.tile([CH_TILE, 2 * PX_CHUNK], f32, tag="ps4",
                                name=f"psd{i}")
                nc.tensor.matmul(psd[:, 0:PX_CHUNK], dmw[:], dmx[:],
                                 skip_group_check=True)

            for _ in range(reps):
                # ---- early layers 2->16->16->4 (lrelu via ACT Prelu with
                # alpha=0.1) + tile-0 layer 4, software-pipelined with
                # skew 1 over 1024-px groups: PE batches each stage while
                # ACT runs one stage behind, and tile-0 stores issue while
                # later groups are still in flight. fp32r matmul output
                # must start at partition 0, so chunks sit in COLUMNS of
                # 2-bank psum tiles.
                lay = (
                    (x1, None, in1_t[:, NPIX : NPIX + 17], 17),
                    (x2, x1, w23_t[:, 0:17], 17),
                    (x3, x2, w23_t[:, 17:22], 5),
                )
                seg = N_CHUNK // STORE_SPLIT
                st0 = stg.tile([CH_TILE, NPIX], f16, tag="st", name="st0")
                # progressive pixel groups: 4x512 first so tile-0's first
                # store only waits on small Prelus, then 2x1024
                GRP = [(0, 512), (512, 512), (1024, 512), (1536, 512),
                       (2048, 1024), (3072, 1024)]
                ST0 = {1: (0, 1024), 3: (1024, 1024), 5: (2048, 2048)}
                NG = len(GRP)

                def early(snum, g):
                    xo, xi, wap, rows = lay[snum]
                    g0, gw = GRP[g]
                    pse = ps4p.tile([CH_TILE, EC], f32, tag="ps4",
                                    name=f"pse{snum}_{g}")
                    for h in range(gw // PX_CHUNK):
                        cs = slice(g0 + h * PX_CHUNK,
                                   g0 + (h + 1) * PX_CHUNK)
                        rhs = in1_t[:, cs] if xi is None else xi[:, cs]
                        nc.tensor.matmul(
                            pse[0:rows, h * PX_CHUNK : (h + 1) * PX_CHUNK],
                            wap, rhs, skip_group_check=True)
                    nc.scalar.activation(xo[:, g0 : g0 + gw],
                                         pse[0:rows, 0:gw],
                                         AF.Prelu, alpha=NEG_SLOPE)

                def l4tile0(g):
                    g0, gw = GRP[g]
                    ps = ps4p.tile([CH_TILE, EC], f32, tag="ps4",
                                   name=f"ps40_{g}")
                    for h in range(gw // PX_CHUNK):
                        cs = slice(g0 + h * PX_CHUNK,
                                   g0 + (h + 1) * PX_CHUNK)
                        nc.tensor.matmul(
                            ps[:, h * PX_CHUNK : (h + 1) * PX_CHUNK],
                            w4_t[:, 0:CH_TILE], x3[:, cs],
                            skip_group_check=True)
                    nc.vector.tensor_copy(st0[:, g0 : g0 + gw],
                                          ps[:, 0:gw])
                    if g in ST0:
                        s0, sw = ST0[g]
                        nc.sync.dma_start(out_d[0:CH_TILE, s0 : s0 + sw],
                                          st0[:, s0 : s0 + sw])

                defer = []
                for step in range(4 + NG - 1):
                    for snum in range(4):
                        g = step - snum
                        if 0 <= g < NG:
                            if g >= 4:
                                defer.append((snum, g))
                            elif snum < 3:
                                early(snum, g)
                            else:
                                l4tile0(g)

                # ---- layer 4, tiles 1-15 (tail of the prologue wavefront
                # threaded between the first tiles so the PE never blocks)
                evac = 0
                for t in range(1, N_TILE):
                    if defer:
                        snum, e = defer.pop(0)
                        if snum < 3:
                            early(snum, e)
                        else:
                            l4tile0(e)
                    ts = slice(t * CH_TILE, (t + 1) * CH_TILE)
                    st = stg.tile([CH_TILE, NPIX], f16, tag="st")
                    for cc in range(N_CHUNK // 2):
                        ps = ps4p.tile([CH_TILE, 2 * PX_CHUNK], f32,
                                       tag="ps4")
                        for h in range(2):
                            c = 2 * cc + h
                            cs = slice(c * PX_CHUNK, (c + 1) * PX_CHUNK)
                            nc.tensor.matmul(
                                ps[:, h * PX_CHUNK : (h + 1) * PX_CHUNK],
                                w4_t[:, ts], x3[:, cs],
                                skip_group_check=True)
                        cs2 = slice(2 * cc * PX_CHUNK,
                                    (2 * cc + 2) * PX_CHUNK)
                        eng = nc.scalar if (evac * 28) // 60 != \
                            ((evac - 1) * 28) // 60 else nc.vector
                        evac += 1
                        if eng is nc.scalar:
                            eng.copy(st[:, cs2], ps[:])
                        else:
                            eng.tensor_copy(st[:, cs2], ps[:])
                        if (2 * cc + 2) % seg == 0:
                            ss = slice((2 * cc + 2 - seg) * PX_CHUNK,
                                       (2 * cc + 2) * PX_CHUNK)
                            nc.sync.dma_start(out_d[ts, ss], st[:, ss])

    nc.compile()
    return nc


def _get_nc():
    if "nc" not in _COMPILED:
        _COMPILED["nc"] = _build_nc()
    return _COMPILED["nc"]


def _make_in_maps(pos, w1, b1, w2, b2, w3, b3, w4, b4):
    f32 = np.float32

    # in1: pos rows 0-1, ones row 2; cols 4096.. hold w1q [3, 17]
    # (w1.T + bias row, col 16 emits the ones row for the next layer)
    in1 = np.ones((3, NPIX + 17), f32)
    in1[0:2, 0:NPIX] = np.asarray(pos, f32).reshape(2, NPIX)
    w1q = np.zeros((3, 17), f32)
    w1q[0:2, 0:16] = np.asarray(w1, f32).T
    w1q[2, 0:16] = np.asarray(b1, f32)
    w1q[2, 16] = 1.0
    in1[:, NPIX:] = w1q

    # w23: cols 0-16 layer-2 lhsT, cols 17-21 layer-3 lhsT; bias row 16
    # pairs the ones row, last col emits the next ones row
    w23 = np.zeros((17, 22), f32)
    w23[0:16, 0:16] = np.asarray(w2, f32).T
    w23[16, 0:16] = np.asarray(b2, f32)
    w23[16, 16] = 1.0
    w23[0:16, 17:21] = np.asarray(w3, f32).T
    w23[16, 17:21] = np.asarray(b3, f32)
    w23[16, 21] = 1.0

    w4 = np.asarray(w4, f32)
    b4 = np.asarray(b4, f32)

    in_maps = []
    for i in range(N_CORES):
        w4q = np.zeros((5, CPC), f32)
        w4q[0:4] = w4[i * CPC : (i + 1) * CPC, :].T
        w4q[4] = b4[i * CPC : (i + 1) * CPC]
        in_maps.append({"in1": in1, "w23": w23, "w4q": w4q})
    return in_maps


def _run_pjrt_nodonate(nc, in_maps):
    """Execute via PJRT WITHOUT output-buffer donation.

    The stock run_bass_via_pjrt donates pre-zeroed output buffers; on this
    axon terminal the donated path intermittently returns corrupted
    outputs, while the identical executable without donation is reliable.
    The kernel writes every output element, so donation is unnecessary.
    """
    import jax
    from jax.sharding import Mesh, PartitionSpec, NamedSharding
    from jax.experimental.shard_map import shard_map
    from concourse import bass2jax, mybir
    from concourse.bass2jax import _bass_exec_p, partition_id_tensor

    bass2jax.install_neuronx_cc_hook()
    partition_name = (nc.partition_id_tensor.name
                      if nc.partition_id_tensor else None)
    in_names, out_names, out_avals = [], [], []
    for alloc in nc.m.functions[0].allocations:
        if not isinstance(alloc, mybir.MemoryLocationSet):
            continue
        name = alloc.memorylocations[0].name
        if alloc.kind == "ExternalInput":
            if name != partition_name:
                in_names.append(name)
        elif alloc.kind == "ExternalOutput":
            out_names.append(name)
            out_avals.append(jax.core.ShapedArray(
                tuple(alloc.tensor_shape), mybir.dt.np(alloc.dtype)))
    n_params = len(in_names)
    all_in = list(in_names) + list(out_names)
    if partition_name is not None:
        all_in.append(partition_name)

    def _body(*args):
        operands = list(args)
        if partition_name is not None:
            operands.append(partition_id_tensor())
        return tuple(_bass_exec_p.bind(
            *operands, out_avals=tuple(out_avals), in_names=tuple(all_in),
            out_names=tuple(out_names), lowering_input_output_aliases=(),
            sim_require_finite=True, sim_require_nnan=True, nc=nc))

    n = len(in_maps)
    mesh = Mesh(np.asarray(jax.devices()[:n]), ("core",))
    spec = (PartitionSpec("core"),)
    fn = jax.jit(shard_map(_body, mesh=mesh,
                           in_specs=spec * (n_params + len(out_names)),
                           out_specs=spec * len(out_names), check_rep=False),
                 keep_unused=True)
    args = [np.concatenate([np.asarray(m[nm]) for m in in_maps], axis=0)
            for nm in in_names]
    args += [np.zeros((n * av.shape[0], *av.shape[1:]), av.dtype)
             for av in out_avals]
    outs = fn(*args)
    return [
        {nm: np.asarray(outs[i]).reshape(n, *out_avals[i].shape)[c]
         for i, nm in enumerate(out_names)}
        for c in range(n)
    ]


def run(trace=False, tmpdir=None, **inputs):
    nc = _get_nc()
    in_maps = _make_in_maps(**inputs)
    full = results = None
    for _ in range(4):
        results = _run_pjrt_nodonate(nc, in_maps)
        parts = [np.asarray(results[i]["out"]) for i in range(N_CORES)]
        full = np.concatenate(parts, axis=0).reshape(
            CH_OUT, CH_IN, X_DIM, Y_DIM).astype(np.float32)
        # guard against the flaky first NEFF execution on a freshly
        # attached terminal (first call occasionally returns garbage on
        # some cores; subsequent executions are correct)
        if _check_row(inputs, full) < 5e-3:
            break

    class _Res:
        exec_time_ns = None
        results = None
    r = _Res()
    r.results = results
    return full, r


def _check_row(inputs, full):
    # cheap host recomputation of output channel (0,0): guards against a
    # flaky first NEFF execution on a freshly-attached terminal (observed:
    # first call occasionally returns garbage; later calls are correct)
    def lrelu(v):
        return np.where(v >= 0, v, NEG_SLOPE * v)

    pos = np.asarray(inputs["pos"], np.float32).reshape(2, NPIX)
    x = lrelu(inputs["w1"] @ pos + np.asarray(inputs["b1"])[:, None])
    x = lrelu(inputs["w2"] @ x + np.asarray(inputs["b2"])[:, None])
    x = lrelu(inputs["w3"] @ x + np.asarray(inputs["b3"])[:, None])
    w4 = np.asarray(inputs["w4"])
    b4 = np.asarray(inputs["b4"])
    outf = full.reshape(NUM_C, NPIX)
    # one sampled row per 128-channel tile on every core (vectorized)
    rows = w4[::CH_TILE] @ x + b4[::CH_TILE][:, None]
    denom = np.maximum(np.max(np.abs(rows), axis=1, keepdims=True), 1e-6)
    return float(np.max(np.abs(outf[::CH_TILE] - rows) / denom))


def kernel(**inputs: np.ndarray) -> np.ndarray:
    out, _ = run(trace=False, **inputs)
    return out


# revision 13
# speedup vs baseline: 1.0432x; 1.0280x over previous
"""Optimized Trainium2 Bass kernel for nn_KernelNet2d (dense_mlp, memory regime).

Network: pos(1,2,64,64) -> 1x1convs 2->16->16->4->16384, leaky_relu(0.1)
between layers, output reshaped to (128,128,64,64).

Sharding: conv4's 16384 output channels split across 8 cores (2048 each);
early layers replicated.

Key optimizations over the fp32 baseline (~188us -> ~65us simulated):
  - Output stored as fp16 (rel err ~5e-4 vs fp32 reference): halves HBM
    write traffic 32MB -> 16MB per core, the dominant cost.
  - ALL matmuls in fp32r (TF32-style PE mode): 1 cycle/row at N>=256 vs
    4 cycles/row for plain fp32; layer 4 is a single K=5 matmul
    (4 weights + folded bias) per (128ch x 512px) tile, ~2e-4 accurate.
    fp32r output must start at partition 0, so early-layer pixel chunks
    sit in COLUMNS of multi-bank psum tiles.
  - Leaky-relu as ONE ACT op: activation(Prelu, alpha=0.1) reading PSUM
    and writing f32r directly (the Lrelu LUT hardcodes slope 0.01, and
    scalar_tensor_tensor fails BIR verification in this walrus build).
  - No memsets: biases fold in as weight rows against a ones row, and
    each layer re-emits the next ones row via an extra weight column
    (ones survive lrelu); inputs packed into 3 DRAM tensors to cut
    serialized DMA-issue latency at the start.
  - Dummy matmuls during the input loads hold the PE p-state ramp so the
    first real matmul runs at full clock.
  - Early layers software-pipelined (skew-1 wavefront over progressive
    512/1024-px groups) with tile-0's layer-4 interleaved, so the first
    HBM store issues at ~12us; the wavefront tail threads between the
    remaining tiles so the in-order PE never blocks.
  - PSUM evacuation [128,1024] copies alternate DVE/ACT (gpsimd cannot
    read PSUM on TRN2); quarter-tile stores from the idle SP queue keep
    the store stream dense (HWDGE generation ~625ns/DMA caps the count).
"""

import numpy as np

X_DIM, Y_DIM = 64, 64
NPIX = X_DIM * Y_DIM          # 4096
CH_OUT, CH_IN = 128, 128
NUM_C = CH_OUT * CH_IN        # 16384
N_CORES = 8
CPC = NUM_C // N_CORES        # 2048 channels per core
NEG_SLOPE = 0.1

PX_CHUNK = 512
N_CHUNK = NPIX // PX_CHUNK    # 8
N_GRP = 2                     # chunk groups of 4 (one per PE quadrant)
CH_TILE = 128
N_TILE = CPC // CH_TILE       # 16
STORE_SPLIT = 4               # stores per output tile

_COMPILED = {}


def _build_nc(reps=1):
    from concourse import bacc, mybir, tile

    f32 = mybir.dt.float32
    f32r = mybir.dt.float32r
    f16 = mybir.dt.float16
    AF = mybir.ActivationFunctionType

    nc = bacc.Bacc(trn_type="TRN2", target_bir_lowering=False)

    # pos + ones row + layer-1 weights packed in one tensor; layer-2/3
    # weights in another: fewer serialized input DMAs at start.
    in1_d = nc.dram_tensor("in1", [3, NPIX + 17], f32r, kind="ExternalInput")
    w23_d = nc.dram_tensor("w23", [17, 22], f32r, kind="ExternalInput")
    w4q_d = nc.dram_tensor("w4q", [5, CPC], f32r, kind="ExternalInput")
    out_d = nc.dram_tensor("out", [CPC, NPIX], f16, kind="ExternalOutput")

    EC = 2 * PX_CHUNK            # early-layer psum tile width (2 banks)
    N_EG = NPIX // EC            # 4 psum groups per early layer

    with tile.TileContext(nc) as tc:
        with (
            tc.tile_pool(name="persist", bufs=1) as per,
            tc.tile_pool(name="ps4", bufs=4, space="PSUM") as ps4p,
            tc.tile_pool(name="stage", bufs=4) as stg,
        ):
            in1_t = per.tile([3, NPIX + 17], f32r)
            w23_t = per.tile([17, 22], f32r)
            w4_t = per.tile([5, CPC], f32r)
            dmw = per.tile([1, 128], f32r)
            dmx = per.tile([1, PX_CHUNK], f32r)
            dm32 = per.tile([1, PX_CHUNK], f32)
            x1 = per.tile([17, NPIX], f32r)   # rows 0-15 acts, row 16 ones
            x2 = per.tile([17, NPIX], f32r)
            x3 = per.tile([5, NPIX], f32r)    # rows 0-3 acts, row 4 ones

            nc.sync.dma_start(in1_t[:], in1_d[:])
            nc.sync.dma_start(w23_t[:], w23_d[:])
            nc.sync.dma_start(w4_t[:], w4q_d[:])

            # warm up the PE during the input loads: garbage matmuls hold
            # the p-state ramp so the first real matmul runs at full clock
            nc.vector.memset(dm32[:], 0.0)
            nc.vector.tensor_copy(dmw[:], dm32[:, 0:128])
            nc.vector.tensor_copy(dmx[:], dm32[:])
            for i in range(4):
                psd = ps4p.tile([CH_TILE, 2 * PX_CHUNK], f32, tag="ps4",
                                name=f"psd{i}")
                nc.tensor.matmul(psd[:, 0:PX_CHUNK], dmw[:], dmx[:],
                                 skip_group_check=True)

            for _ in range(reps):
                # ---- early layers 2->16->16->4 (lrelu via ACT Prelu with
                # alpha=0.1) + tile-0 layer 4, software-pipelined with
                # skew 1 over 1024-px groups: PE batches each stage while
                # ACT runs one stage behind, and tile-0 stores issue while
                # later groups are still in flight. fp32r matmul output
                # must start at partition 0, so chunks sit in COLUMNS of
                # 2-bank psum tiles.
                lay = (
                    (x1, None, in1_t[:, NPIX : NPIX + 17], 17),
                    (x2, x1, w23_t[:, 0:17], 17),
                    (x3, x2, w23_t[:, 17:22], 5),
                )
                seg = N_CHUNK // STORE_SPLIT
                st0 = stg.tile([CH_TILE, NPIX], f16, tag="st", name="st0")
                # progressive pixel groups: 4x512 first so tile-0's first
                # store only waits on small Prelus, then 2x1024
                GRP = [(0, 512), (512, 512), (1024, 512), (1536, 512),
                       (2048, 1024), (3072, 1024)]
                ST0 = {1: (0, 1024), 3: (1024, 1024), 5: (2048, 2048)}
                NG = len(GRP)

                def early(snum, g):
                    xo, xi, wap, rows = lay[snum]
                    g0, gw = GRP[g]
                    pse = ps4p.tile([CH_TILE, EC], f32, tag="ps4",
                                    name=f"pse{snum}_{g}")
                    for h in range(gw // PX_CHUNK):
                        cs = slice(g0 + h * PX_CHUNK,
                                   g0 + (h + 1) * PX_CHUNK)
                        rhs = in1_t[:, cs] if xi is None else xi[:, cs]
                        nc.tensor.matmul(
                            pse[0:rows, h * PX_CHUNK : (h + 1) * PX_CHUNK],
                            wap, rhs, skip_group_check=True)
                    nc.scalar.activation(xo[:, g0 : g0 + gw],
                                         pse[0:rows, 0:gw],
                                         AF.Prelu, alpha=NEG_SLOPE)

                def l4tile0(g):
                    g0, gw = GRP[g]
                    ps = ps4p.tile([CH_TILE, EC], f32, tag="ps4",
                                   name=f"ps40_{g}")
                    for h in range(gw // PX_CHUNK):
                        cs = slice(g0 + h * PX_CHUNK,
                                   g0 + (h + 1) * PX_CHUNK)
                        nc.tensor.matmul(
                            ps[:, h * PX_CHUNK : (h + 1) * PX_CHUNK],
                            w4_t[:, 0:CH_TILE], x3[:, cs],
                            skip_group_check=True)
                    nc.vector.tensor_copy(st0[:, g0 : g0 + gw],
                                          ps[:, 0:gw])
                    if g in ST0:
                        s0, sw = ST0[g]
                        nc.sync.dma_start(out_d[0:CH_TILE, s0 : s0 + sw],
                                          st0[:, s0 : s0 + sw])

                defer = []
                for step in range(4 + NG - 1):
                    for snum in range(4):
                        g = step - snum
                        if 0 <= g < NG:
                            if g >= 4:
                                defer.append((snum, g))
                            elif snum < 3:
                                early(snum, g)
                            else:
                                l4tile0(g)

                # ---- layer 4, tiles 1-15 (tail of the prologue wavefront
                # threaded between the first tiles so the PE never blocks)
                evac = 0
                for t in range(1, N_TILE):
                    if defer:
                        snum, e = defer.pop(0)
                        if snum < 3:
                            early(snum, e)
                        else:
                            l4tile0(e)
                    ts = slice(t * CH_TILE, (t + 1) * CH_TILE)
                    st = stg.tile([CH_TILE, NPIX], f16, tag="st")
                    for cc in range(N_CHUNK // 2):
                        ps = ps4p.tile([CH_TILE, 2 * PX_CHUNK], f32,
                                       tag="ps4")
                        for h in range(2):
                            c = 2 * cc + h
                            cs = slice(c * PX_CHUNK, (c + 1) * PX_CHUNK)
                            nc.tensor.matmul(
                                ps[:, h * PX_CHUNK : (h + 1) * PX_CHUNK],
                                w4_t[:, ts], x3[:, cs],
                                skip_group_check=True)
                        cs2 = slice(2 * cc * PX_CHUNK,
                                    (2 * cc + 2) * PX_CHUNK)
                        eng = nc.scalar if (evac * 28) // 60 != \
                            ((evac - 1) * 28) // 60 else nc.vector
                        evac += 1
                        if eng is nc.scalar:
                            eng.copy(st[:, cs2], ps[:])
                        else:
                            eng.tensor_copy(st[:, cs2], ps[:])
                        if (2 * cc + 2) % seg == 0:
                            ss = slice((2 * cc + 2 - seg) * PX_CHUNK,
                                       (2 * cc + 2) * PX_CHUNK)
                            nc.sync.dma_start(out_d[ts, ss], st[:, ss])

    nc.compile()
    return nc


def _get_nc():
    if "nc" not in _COMPILED:
        _COMPILED["nc"] = _build_nc()
    return _COMPILED["nc"]


def _make_in_maps(pos, w1, b1, w2, b2, w3, b3, w4, b4):
    f32 = np.float32

    # in1: pos rows 0-1, ones row 2; cols 4096.. hold w1q [3, 17]
    # (w1.T + bias row, col 16 emits the ones row for the next layer)
    in1 = np.ones((3, NPIX + 17), f32)
    in1[0:2, 0:NPIX] = np.asarray(pos, f32).reshape(2, NPIX)
    w1q = np.zeros((3, 17), f32)
    w1q[0:2, 0:16] = np.asarray(w1, f32).T
    w1q[2, 0:16] = np.asarray(b1, f32)
    w1q[2, 16] = 1.0
    in1[:, NPIX:] = w1q

    # w23: cols 0-16 layer-2 lhsT, cols 17-21 layer-3 lhsT; bias row 16
    # pairs the ones row, last col emits the next ones row
    w23 = np.zeros((17, 22), f32)
    w23[0:16, 0:16] = np.asarray(w2, f32).T
    w23[16, 0:16] = np.asarray(b2, f32)
    w23[16, 16] = 1.0
    w23[0:16, 17:21] = np.asarray(w3, f32).T
    w23[16, 17:21] = np.asarray(b3, f32)
    w23[16, 21] = 1.0

    w4 = np.asarray(w4, f32)
    b4 = np.asarray(b4, f32)

    in_maps = []
    for i in range(N_CORES):
        w4q = np.zeros((5, CPC), f32)
        w4q[0:4] = w4[i * CPC : (i + 1) * CPC, :].T
        w4q[4] = b4[i * CPC : (i + 1) * CPC]
        in_maps.append({"in1": in1, "w23": w23, "w4q": w4q})
    return in_maps


def _run_pjrt_nodonate(nc, in_maps):
    """Execute via PJRT WITHOUT output-buffer donation.

    The stock run_bass_via_pjrt donates pre-zeroed output buffers; on this
    axon terminal the donated path intermittently returns corrupted
    outputs, while the identical executable without donation is reliable.
    The kernel writes every output element, so donation is unnecessary.
    """
    import jax
    from jax.sharding import Mesh, PartitionSpec, NamedSharding
    from jax.experimental.shard_map import shard_map
    from concourse import bass2jax, mybir
    from concourse.bass2jax import _bass_exec_p, partition_id_tensor

    bass2jax.install_neuronx_cc_hook()
    partition_name = (nc.partition_id_tensor.name
                      if nc.partition_id_tensor else None)
    in_names, out_names, out_avals = [], [], []
    for alloc in nc.m.functions[0].allocations:
        if not isinstance(alloc, mybir.MemoryLocationSet):
            continue
        name = alloc.memorylocations[0].name
        if alloc.kind == "ExternalInput":
            if name != partition_name:
                in_names.append(name)
        elif alloc.kind == "ExternalOutput":
            out_names.append(name)
            out_avals.append(jax.core.ShapedArray(
                tuple(alloc.tensor_shape), mybir.dt.np(alloc.dtype)))
    n_params = len(in_names)
    all_in = list(in_names) + list(out_names)
    if partition_name is not None:
        all_in.append(partition_name)

    def _body(*args):
        operands = list(args)
        if partition_name is not None:
            operands.append(partition_id_tensor())
        return tuple(_bass_exec_p.bind(
            *operands, out_avals=tuple(out_avals), in_names=tuple(all_in),
            out_names=tuple(out_names), lowering_input_output_aliases=(),
            sim_require_finite=True, sim_require_nnan=True, nc=nc))

    n = len(in_maps)
    mesh = Mesh(np.asarray(jax.devices()[:n]), ("core",))
    spec = (PartitionSpec("core"),)
    fn = jax.jit(shard_map(_body, mesh=mesh,
                           in_specs=spec * (n_params + len(out_names)),
                           out_specs=spec * len(out_names), check_rep=False),
                 keep_unused=True)
    args = [np.concatenate([np.asarray(m[nm]) for m in in_maps], axis=0)
            for nm in in_names]
    args += [np.zeros((n * av.shape[0], *av.shape[1:]), av.dtype)
             for av in out_avals]
    outs = fn(*args)
    return [
        {nm: np.asarray(outs[i]).reshape(n, *out_avals[i].shape)[c]
         for i, nm in enumerate(out_names)}
        for c in range(n)
    ]


def run(trace=False, tmpdir=None, **inputs):
    nc = _get_nc()
    in_maps = _make_in_maps(**inputs)
    full = results = None
    for _ in range(4):
        results = _run_pjrt_nodonate(nc, in_maps)
        parts = [np.asarray(results[i]["out"]) for i in range(N_CORES)]
        full = np.concatenate(parts, axis=0).reshape(
            CH_OUT, CH_IN, X_DIM, Y_DIM).astype(np.float32)
        # guard against the flaky first NEFF execution on a freshly
        # attached terminal (first call occasionally returns garbage on
        # some cores; subsequent executions are correct)
        if _check_row(inputs, full) < 5e-3:
            break

    class _Res:
        exec_time_ns = None
        results = None
    r = _Res()
    r.results = results
    return full, r


def _check_row(inputs, full):
    # cheap host recomputation of output channel (0,0): guards against a
    # flaky first NEFF execution on a freshly-attached terminal (observed:
    # first call occasionally returns garbage; later calls are correct)
    def lrelu(v):
        return np.where(v >= 0, v, NEG_SLOPE * v)

    pos = np.asarray(inputs["pos"], np.float32).reshape(2, NPIX)
    x = lrelu(inputs["w1"] @ pos + np.asarray(inputs["b1"])[:, None])
    x = lrelu(inputs["w2"] @ x + np.asarray(inputs["b2"])[:, None])
    x = lrelu(inputs["w3"] @ x + np.asarray(inputs["b3"])[:, None])
    w4 = np.asarray(inputs["w4"])
    b4 = np.asarray(inputs["b4"])
    outf = full.reshape(NUM_C, NPIX)
    # one sampled row per 128-channel tile on every core (vectorized)
    rows = w4[::CH_TILE] @ x + b4[::CH_TILE][:, None]
    denom = np.maximum(np.max(np.abs(rows), axis=1, keepdims=True), 1e-6)
    return float(np.max(np.abs(outf[::CH_TILE] - rows) / denom))


def kernel(**inputs: np.ndarray) -> np.ndarray:
    out, _ = run(trace=False, **inputs)
    return out
